# revision 17
# baseline (speedup 1.0000x reference)
"""Trainium2 Bass kernel for Conformer relative-position MHA.

Sharding: 16 (batch, head) pairs across 8 cores -> each core handles one batch
row (b = core//4) and two consecutive heads (2*(core%4), 2*(core%4)+1).
Per-head Dh x Dh projection weights are replicated (with LayerNorm gamma/beta
folded in on the host); the final out-projection is computed per-core against
the matching Wo row-slice and partial outputs are summed on the host.

Rel-shift is realized as a DRAM round-trip of the positional score matrix T:
T is written row-major into a padded (s, s+1) buffer (column 0 zeroed) and the
shifted matrix is exactly the same bytes re-read with row stride s at element
offset s.  Attention probabilities are transposed for the PV matmul with the
DMA xbar transpose (bf16, SBUF->SBUF).
"""

import math
import sys

import numpy as np

for _p in ("/opt/trn_rl_repo",):
    if _p not in sys.path:
        sys.path.insert(0, _p)

import ml_dtypes  # noqa: E402

BF = ml_dtypes.bfloat16

B, S, E, H, Dh = 2, 2048, 1024, 8, 128
NCORES = 8
HPC = 2  # heads per core


def _patch_tile_drain():
    """This toolchain's walrus rejects >1 sem-wait on the kernel-tail Drain
    ("Too many sync wait commands").  Re-emit the tail waits as single-wait
    NOPs on SP before a clean Drain."""
    from concourse import tile
    from concourse.vector_clock import ScopedClock

    if getattr(tile.TileContext, "_drain_split_patch", False):
        return

    def _drain_and_barrier(self, tick_clock, wait_clock):
        nc = self.nc
        probe = nc.sync.nop()
        wait_clock.add_sem_waits(
            probe.ins, ScopedClock({None: tick_clock.global_clock})
        )
        si = probe.ins.sync_info
        waits = list(si.on_wait) if si is not None else []
        probe.ins.sync_info = None
        by_num = {h.num: h for h in self.sems.allocated().values()}
        for w in waits:
            nc.sync.wait_ge(by_num[w.id], w.wait_value)
        nc.sync.drain()
        nc.all_engine_barrier()
        assert self.sems is not None
        popped = nc._tile_sem_poison_stack.pop()
        assert popped is self._sem_poison
        nc.clear_and_free_semaphores(list(self.sems.allocated().values()))
        nc.all_engine_barrier()

    tile.TileContext._drain_and_barrier = _drain_and_barrier
    tile.TileContext._drain_split_patch = True


def _split_bir_waits(bir_bytes, max_waits=1):
    """This toolchain's walrus rejects instructions carrying more than two
    semaphore waits.  Hoist excess waits onto preceding same-engine NoOps."""
    import json as _json

    bir = _json.loads(bir_bytes)
    ctr = 0
    for fn in bir.get("functions", []):
        for blk in fn.get("blocks", []):
            insts = blk.get("instructions")
            if not insts:
                continue
            out = []
            changed = False
            for inst in insts:
                si = inst.get("sync_info")
                waits = (si or {}).get("on_wait") or []
                limit = 1 if inst.get("opcode") in ("Drain", "NoOp") else max_waits
                if len(waits) > limit:
                    extra, keep = waits[:-limit], waits[-limit:]
                    for w in extra:
                        ctr += 1
                        out.append(
                            {
                                "debug": inst.get("debug", 0),
                                "engine": inst["engine"],
                                "ins": [],
                                "outs": [],
                                "name": f"WSPL-{ctr}",
                                "opcode": "NoOp",
                                "sync_info": {"on_update": [], "on_wait": [w]},
                            }
                        )
                    si["on_wait"] = keep
                    changed = True
                out.append(inst)
            if changed:
                blk["instructions"] = out
    return _json.dumps(bir).encode()


def _patch_wait_split():
    from concourse import bass2jax, bass_utils

    if getattr(bass_utils, "_wait_split_patch", False):
        return
    orig = bass_utils.compile_bir_kernel

    def wrapped(bir_str, *a, **k):
        return orig(_split_bir_waits(bir_str), *a, **k)

    bass_utils.compile_bir_kernel = wrapped
    bass2jax.compile_bir_kernel = wrapped
    bass_utils._wait_split_patch = True


def _build_program(s):
    import concourse.bass as bass
    import concourse.mybir as mybir
    from concourse import tile

    _patch_tile_drain()
    _patch_wait_split()

    f32 = mybir.dt.float32
    bf16 = mybir.dt.bfloat16
    AF = mybir.ActivationFunctionType
    OP = mybir.AluOpType

    nt = s // 128  # number of 128-row tiles along the sequence
    nw = min(512, s)
    nn = max(1, s // 512)  # 512-wide chunks along the sequence
    ne = E // 512

    nc = bass.Bass()

    x_in = nc.declare_dram_parameter("x", [s, E], bf16, isOutput=False)
    w_in = nc.declare_dram_parameter("w", [HPC, 4, Dh, Dh], bf16, isOutput=False)
    bqu_in = nc.declare_dram_parameter("bqu", [HPC, Dh], f32, isOutput=False)
    bqv_in = nc.declare_dram_parameter("bqv", [HPC, Dh], f32, isOutput=False)
    bk_in = nc.declare_dram_parameter("bk", [HPC, Dh], f32, isOutput=False)
    bv_in = nc.declare_dram_parameter("bvrow", [HPC, Dh], bf16, isOutput=False)
    pet_in = nc.declare_dram_parameter("peT", [HPC, Dh, s], bf16, isOutput=False)
    wo_in = nc.declare_dram_parameter("wo", [HPC, Dh, E], bf16, isOutput=False)
    id_in = nc.declare_dram_parameter("ident", [Dh, Dh], bf16, isOutput=False)
    out_ext = nc.declare_dram_parameter("out", [s, E], f32, isOutput=True)

    inv_sqrt_e = 1.0 / math.sqrt(float(E))

    with tile.TileContext(nc) as tc:
        with (
            tc.tile_pool(name="psmm", bufs=4, space="PSUM") as psmm,
            tc.tile_pool(name="psctx", bufs=1, space="PSUM") as psctx,
            tc.tile_pool(name="dram", bufs=2, space="DRAM") as drpool,
            tc.tile_pool(name="xp", bufs=4) as xp,
            tc.tile_pool(name="xnp", bufs=3) as xnp,
            tc.tile_pool(name="stat", bufs=8) as statp,
            tc.tile_pool(name="col", bufs=16) as colp,
            tc.tile_pool(name="hbig2", bufs=2) as hbig2,
            tc.tile_pool(name="hbig1", bufs=2) as hbig1,
            tc.tile_pool(name="wsmall", bufs=2) as wsmall,
            tc.tile_pool(name="tst", bufs=3) as tstp,
            tc.tile_pool(name="shp", bufs=3) as shp,
            tc.tile_pool(name="scp", bufs=3) as scp,
            tc.tile_pool(name="att", bufs=1) as attp,
            tc.tile_pool(name="outp", bufs=2) as outp,
        ):
            # ---- constants / per-head small params ----
            ones_row = colp.tile([1, Dh], bf16, tag="ones")
            nc.vector.memset(ones_row[:], 1.0)
            ident_sb = wsmall.tile([Dh, Dh], bf16, tag="ident", bufs=1)
            nc.scalar.dma_start(ident_sb[:], id_in[:])

            wq_sb, wk_sb, wv_sb, wp_sb = [], [], [], []
            bqu_sb, bqv_sb, bk_sb, bvr_sb = [], [], [], []
            wo_sb = []
            for h in range(HPC):
                wt = wsmall.tile([Dh, 4, Dh], bf16, tag="w4")
                nc.scalar.dma_start(wt[:], w_in[h].rearrange("j d e -> d j e"))
                wq_sb.append(wt[:, 0, :])
                wk_sb.append(wt[:, 1, :])
                wv_sb.append(wt[:, 2, :])
                wp_sb.append(wt[:, 3, :])
                for lst, src, tg in (
                    (bqu_sb, bqu_in, "bqu"),
                    (bqv_sb, bqv_in, "bqv"),
                    (bk_sb, bk_in, "bk"),
                ):
                    t = colp.tile([Dh, 1], f32, tag=tg)
                    nc.scalar.dma_start(t[:], src[h])
                    lst.append(t)
                t = colp.tile([1, Dh], bf16, tag="bvr")
                nc.scalar.dma_start(t[:], bv_in[h])
                bvr_sb.append(t)
                wo_t = hbig1.tile([Dh, E], bf16, tag="wo")
                nc.scalar.dma_start(wo_t[:], wo_in[h])
                wo_sb.append(wo_t)

            # ---- LayerNorm + head-slice transpose ----
            xhT = hbig2.tile([Dh, HPC, s], bf16, tag="xhT", bufs=1)
            for st in range(nt):
                x_t = xp.tile([128, E], bf16)
                nc.sync.dma_start(x_t[:], x_in[st * 128 : (st + 1) * 128, :])
                stats = statp.tile([128, 2, 6], f32)
                for j in range(2):
                    nc.vector.bn_stats(stats[:, j, :], x_t[:, j * 512 : (j + 1) * 512])
                ag = statp.tile([128, 2], f32, tag="ag")
                nc.vector.bn_aggr(ag[:], stats[:])
                veps = colp.tile([128, 1], f32, tag="veps")
                nc.vector.tensor_scalar_add(veps[:], ag[:, 1:2], 1e-5)
                rvar = colp.tile([128, 1], f32, tag="rvar")
                nc.vector.reciprocal(rvar[:], veps[:])
                rstd = colp.tile([128, 1], f32, tag="rstd")
                nc.scalar.sqrt(rstd[:], rvar[:])
                xc_t = xnp.tile([128, HPC * Dh], bf16, tag="xc")
                nc.vector.tensor_scalar_sub(xc_t[:], x_t[:, : HPC * Dh], ag[:, 0:1])
                xn_t = xnp.tile([128, HPC * Dh], bf16)
                nc.scalar.mul(xn_t[:], xc_t[:], rstd[:])
                nc.scalar.dma_start_transpose(
                    xhT[:, :, st * 128 : (st + 1) * 128], xn_t[:]
                )

            ctxT_sb = []
            for h in range(HPC):
                # ---- projections ----
                peT_t = hbig1.tile([Dh, s], bf16, tag="peT", bufs=1)
                nc.scalar.dma_start(peT_t[:], pet_in[h])

                quT = hbig2.tile([Dh, s], bf16, tag="quT")
                qvT = hbig1.tile([Dh, s], bf16, tag="qvT", bufs=1)
                kT = hbig2.tile([Dh, s], bf16, tag="kT")
                pT = hbig1.tile([Dh, s], bf16, tag="pT", bufs=1)
                v_sb = hbig2.tile([Dh, s], bf16, tag="v")

                for j in range(nn):
                    csl = slice(j * nw, (j + 1) * nw)
                    ps = psmm.tile([128, nw], f32)
                    nc.tensor.matmul(ps[:], wq_sb[h], xhT[:, h, csl])
                    nc.vector.tensor_scalar_add(quT[:, csl], ps[:], bqu_sb[h][:])
                    nc.scalar.add(qvT[:, csl], ps[:], bqv_sb[h][:])
                    ps = psmm.tile([128, nw], f32)
                    nc.tensor.matmul(ps[:], wk_sb[h], xhT[:, h, csl])
                    nc.vector.tensor_scalar_add(kT[:, csl], ps[:], bk_sb[h][:])
                    ps = psmm.tile([128, nw], f32)
                    nc.tensor.matmul(ps[:], wp_sb[h], peT_t[:, csl])
                    nc.scalar.copy(pT[:, csl], ps[:])
                # v in natural layout: v[s_tile, d] = xh @ Wv + bv
                for g in range(max(1, nt // 4)):
                    ps = psmm.tile([128, nw], f32)
                    gnt = min(4, nt)
                    for j in range(gnt):
                        st = g * 4 + j
                        nc.tensor.matmul(
                            ps[:, j * 128 : (j + 1) * 128],
                            xhT[:, h, st * 128 : (st + 1) * 128],
                            wv_sb[h],
                            start=True,
                            stop=False,
                        )
                        nc.tensor.matmul(
                            ps[:, j * 128 : (j + 1) * 128],
                            ones_row[:],
                            bvr_sb[h][:],
                            start=False,
                            stop=True,
                        )
                    nc.vector.tensor_copy(
                        v_sb[:, g * nw : g * nw + gnt * 128], ps[:, : gnt * 128]
                    )

                # ---- phase A: T = (q + v_bias) @ p^T  -> padded DRAM buffer ----
                tdram = drpool.tile([s, s + 1], bf16, tag="tshift")
                tflat = tdram[:].rearrange("a b -> (a b)")
                shview = tflat[s : s + s * s].rearrange("(a b) -> a b", b=s)
                for qt in range(nt):
                    tst = tstp.tile([128, s + 1], bf16)
                    nc.gpsimd.memset(tst[:, 0:1], 0.0)
                    for j in range(nn):
                        ps = psmm.tile([128, nw], f32)
                        nc.tensor.matmul(
                            ps[:], qvT[:, qt * 128 : (qt + 1) * 128], pT[:, j * nw : (j + 1) * nw]
                        )
                        nc.vector.tensor_copy(tst[:, 1 + j * nw : 1 + (j + 1) * nw], ps[:])
                    nc.sync.dma_start(tdram[qt * 128 : (qt + 1) * 128, :], tst[:])

                # ---- phase B: scores + softmax + transpose ----
                attnT = attp.tile([128, nt, s], bf16, tag="attnT")
                for qt in range(nt):
                    sh = shp.tile([128, s], bf16)
                    nc.sync.dma_start(sh[:], shview[qt * 128 : (qt + 1) * 128, :])
                    sc = scp.tile([128, s], bf16)
                    pss = []
                    for j in range(nn):
                        ps = psmm.tile([128, nw], f32, tag="ps", name=f"ps_{h}_{qt}_{j}")
                        nc.tensor.matmul(
                            ps[:],
                            quT[:, qt * 128 : (qt + 1) * 128],
                            kT[:, j * nw : (j + 1) * nw],
                            start=True,
                            stop=False,
                        )
                        pss.append(ps)
                    rsa = colp.tile([128, 4], f32, tag="rsa")
                    for j in range(nn):
                        nc.tensor.matmul(
                            pss[j][:],
                            ident_sb[:],
                            sh[:, j * nw : (j + 1) * nw],
                            start=False,
                            stop=True,
                        )
                        nc.scalar.activation(
                            sc[:, j * nw : (j + 1) * nw],
                            pss[j][:],
                            AF.Exp,
                            scale=inv_sqrt_e,
                            accum_out=rsa[:, j : j + 1],
                        )
                    rs = colp.tile([128, 1], f32, tag="rs")
                    nc.vector.reduce_sum(rs[:], rsa[:, :nn], axis=mybir.AxisListType.X)
                    rr = colp.tile([128, 1], f32, tag="rr")
                    nc.vector.reciprocal(rr[:], rs[:])
                    nc.vector.tensor_scalar_mul(sc[:], sc[:], rr[:])
                    nc.sync.dma_start_transpose(
                        attnT[:, :, qt * 128 : (qt + 1) * 128], sc[:]
                    )

                # ---- phase PV: ctxT = v^T @ attn^T ----
                ctx_ps = psctx.tile([128, s], f32)
                for kc in range(nt):
                    for j in range(nn):
                        nc.tensor.matmul(
                            ctx_ps[:, j * nw : (j + 1) * nw],
                            v_sb[:, kc * 128 : (kc + 1) * 128],
                            attnT[:, kc, j * nw : (j + 1) * nw],
                            start=(kc == 0),
                            stop=(kc == nt - 1),
                        )
                ctxT = hbig2.tile([Dh, s], bf16, tag="ctxT")
                nc.vector.tensor_copy(ctxT[:], ctx_ps[:])
                ctxT_sb.append(ctxT)

            # ---- out-projection: out[s,:] = sum_h ctxT_h^T @ Wo_h ----
            for st in range(nt):
                out_t = outp.tile([128, E], f32)
                for j in range(ne):
                    ops = psmm.tile([128, 512], f32, tag="ps")
                    for h in range(HPC):
                        nc.tensor.matmul(
                            ops[:],
                            ctxT_sb[h][:, st * 128 : (st + 1) * 128],
                            wo_sb[h][:, j * 512 : (j + 1) * 512],
                            start=(h == 0),
                            stop=(h == HPC - 1),
                        )
                    nc.scalar.copy(out_t[:, j * 512 : (j + 1) * 512], ops[:])
                nc.sync.dma_start(out_ext[st * 128 : (st + 1) * 128, :], out_t[:])

    return nc


def _pos_encoding(s, e):
    pos = np.arange(s, dtype=np.float32)[:, None]
    div = np.exp(
        np.arange(0, e, 2, dtype=np.float32) * (-math.log(10000.0) / e)
    ).astype(np.float32)
    pe = np.zeros((s, e), dtype=np.float32)
    pe[:, 0::2] = np.sin(pos * div)
    pe[:, 1::2] = np.cos(pos * div)
    return pe


def make_core_inputs(inputs, s=S):
    """Host-side prep: fold LN gamma/beta into per-head weights, permute the
    input feature blocks so each core's two heads occupy columns 0:256."""
    x = np.asarray(inputs["inputs"], np.float32)
    gam = np.asarray(inputs["ln_gamma"], np.float32)
    bet = np.asarray(inputs["ln_beta"], np.float32)
    Wq = np.asarray(inputs["Wq"], np.float32)
    bq = np.asarray(inputs["bq"], np.float32)
    Wk = np.asarray(inputs["Wk"], np.float32)
    bk = np.asarray(inputs["bk"], np.float32)
    Wv = np.asarray(inputs["Wv"], np.float32)
    bv = np.asarray(inputs["bv"], np.float32)
    Wp = np.asarray(inputs["Wp"], np.float32)
    u_bias = np.asarray(inputs["u_bias"], np.float32)
    v_bias = np.asarray(inputs["v_bias"], np.float32)

    pe = _pos_encoding(s, E).reshape(s, H, Dh)

    in_maps = []
    Dh_ = Dh
    for c in range(NCORES):
        b = c // (NCORES // B)
        h0 = HPC * (c % (NCORES // B))
        heads = [h0 + i for i in range(HPC)]
        order = heads + [j for j in range(H) if j not in heads]
        x_c = np.ascontiguousarray(
            x[b, :s].reshape(s, H, Dh)[:, order, :].reshape(s, E)
        ).astype(BF)
        w4 = np.zeros((HPC, 4, Dh, Dh), BF)
        bqu = np.zeros((HPC, Dh), np.float32)
        bqv = np.zeros((HPC, Dh), np.float32)
        bkk = np.zeros((HPC, Dh), np.float32)
        bvr = np.zeros((HPC, Dh), BF)
        peT = np.zeros((HPC, Dh, s), BF)
        wo = np.zeros((HPC, Dh, E), BF)
        for i, hh in enumerate(heads):
            g = gam[hh * Dh : (hh + 1) * Dh]
            be = bet[hh * Dh : (hh + 1) * Dh]
            w4[i, 0] = (g[:, None] * Wq).astype(BF)
            w4[i, 1] = (g[:, None] * Wk).astype(BF)
            w4[i, 2] = (g[:, None] * Wv).astype(BF)
            w4[i, 3] = Wp.astype(BF)
            bqp = bq + be @ Wq
            bkp = bk + be @ Wk
            bvp = bv + be @ Wv
            bqu[i] = bqp + u_bias[hh]
            bqv[i] = bqp + v_bias[hh]
            bkk[i] = bkp
            bvr[i] = bvp.astype(BF)
            peT[i] = pe[:, hh, :].T.astype(BF)
            wo[i] = np.asarray(inputs["Wo"], np.float32)[
                hh * Dh : (hh + 1) * Dh, :
            ].astype(BF)
        in_maps.append(
            {
                "x": x_c,
                "ident": np.eye(Dh, dtype=BF),
                "w": w4,
                "bqu": bqu,
                "bqv": bqv,
                "bk": bkk,
                "bvrow": bvr,
                "peT": peT,
                "wo": wo,
            }
        )
    return in_maps


_PROG = {}


def _program(s):
    if s not in _PROG:
        _PROG[s] = _build_program(s)
    return _PROG[s]


def kernel(**inputs):
    from concourse.bass_utils import run_bass_kernel_spmd

    nc = _program(S)
    in_maps = make_core_inputs(inputs, S)
    res = run_bass_kernel_spmd(nc, in_maps, list(range(NCORES)))
    kernel.last_results = res
    out = np.zeros((B, S, E), np.float32)
    for c in range(NCORES):
        out[c // (NCORES // B)] += np.asarray(res.results[c]["out"], np.float32)
    out += np.asarray(inputs["bo"], np.float32)
    return out


# revision 19
# speedup vs baseline: 1.0407x; 1.0407x over previous
"""Trainium2 Bass kernel for Conformer relative-position MHA.

Sharding: 16 (batch, head) pairs across 8 cores -> each core handles one batch
row (b = core//4) and two consecutive heads (2*(core%4), 2*(core%4)+1).
Per-head Dh x Dh projection weights are replicated (with LayerNorm gamma/beta
folded in on the host); the final out-projection is computed per-core against
the matching Wo row-slice and partial outputs are summed on the host.

Rel-shift is realized as a DRAM round-trip of the positional score matrix T:
T is written row-major into a padded (s, s+1) buffer (column 0 zeroed) and the
shifted matrix is exactly the same bytes re-read with row stride s at element
offset s.  Attention probabilities are transposed for the PV matmul with the
DMA xbar transpose (bf16, SBUF->SBUF).
"""

import math
import sys

import numpy as np

for _p in ("/opt/trn_rl_repo",):
    if _p not in sys.path:
        sys.path.insert(0, _p)

import ml_dtypes  # noqa: E402

BF = ml_dtypes.bfloat16

B, S, E, H, Dh = 2, 2048, 1024, 8, 128
NCORES = 8
HPC = 2  # heads per core


def _patch_tile_drain():
    """This toolchain's walrus rejects >1 sem-wait on the kernel-tail Drain
    ("Too many sync wait commands").  Re-emit the tail waits as single-wait
    NOPs on SP before a clean Drain."""
    from concourse import tile
    from concourse.vector_clock import ScopedClock

    if getattr(tile.TileContext, "_drain_split_patch", False):
        return

    def _drain_and_barrier(self, tick_clock, wait_clock):
        nc = self.nc
        probe = nc.sync.nop()
        wait_clock.add_sem_waits(
            probe.ins, ScopedClock({None: tick_clock.global_clock})
        )
        si = probe.ins.sync_info
        waits = list(si.on_wait) if si is not None else []
        probe.ins.sync_info = None
        by_num = {h.num: h for h in self.sems.allocated().values()}
        for w in waits:
            nc.sync.wait_ge(by_num[w.id], w.wait_value)
        nc.sync.drain()
        nc.all_engine_barrier()
        assert self.sems is not None
        popped = nc._tile_sem_poison_stack.pop()
        assert popped is self._sem_poison
        nc.clear_and_free_semaphores(list(self.sems.allocated().values()))
        nc.all_engine_barrier()

    tile.TileContext._drain_and_barrier = _drain_and_barrier
    tile.TileContext._drain_split_patch = True


def _split_bir_waits(bir_bytes, max_waits=1):
    """This toolchain's walrus rejects instructions carrying more than two
    semaphore waits.  Hoist excess waits onto preceding same-engine NoOps."""
    import json as _json

    bir = _json.loads(bir_bytes)
    ctr = 0
    for fn in bir.get("functions", []):
        for blk in fn.get("blocks", []):
            insts = blk.get("instructions")
            if not insts:
                continue
            out = []
            changed = False
            for inst in insts:
                si = inst.get("sync_info")
                waits = (si or {}).get("on_wait") or []
                limit = 1 if inst.get("opcode") in ("Drain", "NoOp") else max_waits
                if len(waits) > limit:
                    extra, keep = waits[:-limit], waits[-limit:]
                    for w in extra:
                        ctr += 1
                        out.append(
                            {
                                "debug": inst.get("debug", 0),
                                "engine": inst["engine"],
                                "ins": [],
                                "outs": [],
                                "name": f"WSPL-{ctr}",
                                "opcode": "NoOp",
                                "sync_info": {"on_update": [], "on_wait": [w]},
                            }
                        )
                    si["on_wait"] = keep
                    changed = True
                out.append(inst)
            if changed:
                blk["instructions"] = out
    return _json.dumps(bir).encode()


def _patch_wait_split():
    from concourse import bass2jax, bass_utils

    if getattr(bass_utils, "_wait_split_patch", False):
        return
    orig = bass_utils.compile_bir_kernel

    def wrapped(bir_str, *a, **k):
        return orig(_split_bir_waits(bir_str), *a, **k)

    bass_utils.compile_bir_kernel = wrapped
    bass2jax.compile_bir_kernel = wrapped
    bass_utils._wait_split_patch = True


def _build_program(s):
    import concourse.bass as bass
    import concourse.mybir as mybir
    from concourse import tile

    _patch_tile_drain()
    _patch_wait_split()

    f32 = mybir.dt.float32
    bf16 = mybir.dt.bfloat16
    AF = mybir.ActivationFunctionType
    OP = mybir.AluOpType

    nt = s // 128  # number of 128-row tiles along the sequence
    nw = min(512, s)
    nn = max(1, s // 512)  # 512-wide chunks along the sequence
    ne = E // 512

    nc = bass.Bass()

    x_in = nc.declare_dram_parameter("x", [s, E], bf16, isOutput=False)
    w_in = nc.declare_dram_parameter("w", [HPC, 4, Dh, Dh], bf16, isOutput=False)
    bqu_in = nc.declare_dram_parameter("bqu", [HPC, Dh], f32, isOutput=False)
    bqv_in = nc.declare_dram_parameter("bqv", [HPC, Dh], f32, isOutput=False)
    bk_in = nc.declare_dram_parameter("bk", [HPC, Dh], f32, isOutput=False)
    bv_in = nc.declare_dram_parameter("bvrow", [HPC, Dh], bf16, isOutput=False)
    pet_in = nc.declare_dram_parameter("peT", [HPC, Dh, s], bf16, isOutput=False)
    wo_in = nc.declare_dram_parameter("wo", [HPC, Dh, E], bf16, isOutput=False)
    id_in = nc.declare_dram_parameter("ident", [Dh, Dh], bf16, isOutput=False)
    out_ext = nc.declare_dram_parameter("out", [s, E], f32, isOutput=True)

    inv_sqrt_e = 1.0 / math.sqrt(float(E))

    with tile.TileContext(nc) as tc:
        with (
            tc.tile_pool(name="psmm", bufs=6, space="PSUM") as psmm,
            tc.tile_pool(name="psctx", bufs=1, space="PSUM") as psctx,
            tc.tile_pool(name="dram", bufs=2, space="DRAM") as drpool,
            tc.tile_pool(name="xp", bufs=4) as xp,
            tc.tile_pool(name="xnp", bufs=3) as xnp,
            tc.tile_pool(name="stat", bufs=8) as statp,
            tc.tile_pool(name="col", bufs=16) as colp,
            tc.tile_pool(name="hbig2", bufs=2) as hbig2,
            tc.tile_pool(name="hbig1", bufs=2) as hbig1,
            tc.tile_pool(name="wsmall", bufs=2) as wsmall,
            tc.tile_pool(name="tst", bufs=3) as tstp,
            tc.tile_pool(name="shp", bufs=3) as shp,
            tc.tile_pool(name="scp", bufs=3) as scp,
            tc.tile_pool(name="att", bufs=1) as attp,
            tc.tile_pool(name="outp", bufs=2) as outp,
        ):
            # ---- constants / per-head small params ----
            ones_row = colp.tile([1, Dh], bf16, tag="ones")
            nc.vector.memset(ones_row[:], 1.0)
            ident_sb = wsmall.tile([Dh, Dh], bf16, tag="ident", bufs=1)
            nc.scalar.dma_start(ident_sb[:], id_in[:])

            wq_sb, wk_sb, wv_sb, wp_sb = [], [], [], []
            bqu_sb, bqv_sb, bk_sb, bvr_sb = [], [], [], []
            wo_sb = []
            for h in range(HPC):
                wt = wsmall.tile([Dh, 4, Dh], bf16, tag="w4")
                nc.scalar.dma_start(wt[:], w_in[h].rearrange("j d e -> d j e"))
                wq_sb.append(wt[:, 0, :])
                wk_sb.append(wt[:, 1, :])
                wv_sb.append(wt[:, 2, :])
                wp_sb.append(wt[:, 3, :])
                for lst, src, tg in (
                    (bqu_sb, bqu_in, "bqu"),
                    (bqv_sb, bqv_in, "bqv"),
                    (bk_sb, bk_in, "bk"),
                ):
                    t = colp.tile([Dh, 1], f32, tag=tg)
                    nc.scalar.dma_start(t[:], src[h])
                    lst.append(t)
                t = colp.tile([1, Dh], bf16, tag="bvr")
                nc.scalar.dma_start(t[:], bv_in[h])
                bvr_sb.append(t)
                wo_t = hbig1.tile([Dh, E], bf16, tag="wo")
                nc.scalar.dma_start(wo_t[:], wo_in[h])
                wo_sb.append(wo_t)

            # ---- LayerNorm + head-slice transpose (band-split for overlap) ----
            bs = min(4, nt)  # s-tiles per band
            nbands = nt // bs
            xhT = [
                hbig2.tile([Dh, HPC, bs * 128], bf16, tag=f"xhT{bb}", bufs=1,
                           name=f"xhT{bb}")
                for bb in range(nbands)
            ]
            for st in range(nt):
                x_t = xp.tile([128, E], bf16)
                nc.sync.dma_start(x_t[:], x_in[st * 128 : (st + 1) * 128, :])
                stats = statp.tile([128, 2, 6], f32)
                for j in range(2):
                    nc.vector.bn_stats(stats[:, j, :], x_t[:, j * 512 : (j + 1) * 512])
                ag = statp.tile([128, 2], f32, tag="ag")
                nc.vector.bn_aggr(ag[:], stats[:])
                veps = colp.tile([128, 1], f32, tag="veps")
                nc.vector.tensor_scalar_add(veps[:], ag[:, 1:2], 1e-5)
                rvar = colp.tile([128, 1], f32, tag="rvar")
                nc.vector.reciprocal(rvar[:], veps[:])
                rstd = colp.tile([128, 1], f32, tag="rstd")
                nc.scalar.sqrt(rstd[:], rvar[:])
                xc_t = xnp.tile([128, HPC * Dh], bf16, tag="xc")
                nc.vector.tensor_scalar_sub(xc_t[:], x_t[:, : HPC * Dh], ag[:, 0:1])
                xn_t = xnp.tile([128, HPC * Dh], bf16)
                nc.scalar.mul(xn_t[:], xc_t[:], rstd[:])
                nc.scalar.dma_start_transpose(
                    xhT[st // bs][:, :, (st % bs) * 128 : (st % bs + 1) * 128],
                    xn_t[:],
                )

            ctxT_sb = []
            for h in range(HPC):
                # ---- projections ----
                peT_t = hbig1.tile([Dh, s], bf16, tag="peT", bufs=1)
                nc.scalar.dma_start(peT_t[:], pet_in[h])

                quT = hbig2.tile([Dh, s], bf16, tag="quT")
                qvT = hbig1.tile([Dh, s], bf16, tag="qvT", bufs=1)
                kT = hbig2.tile([Dh, s], bf16, tag="kT")
                pT = hbig1.tile([Dh, s], bf16, tag="pT", bufs=1)
                v_sb = hbig2.tile([Dh, s], bf16, tag="v")

                for j in range(nn):
                    csl = slice(j * nw, (j + 1) * nw)
                    xb = xhT[min(j, nbands - 1)]
                    ps = psmm.tile([128, nw], f32)
                    nc.tensor.matmul(ps[:], wq_sb[h], xb[:, h, :])
                    nc.vector.tensor_scalar_add(quT[:, csl], ps[:], bqu_sb[h][:])
                    nc.scalar.add(qvT[:, csl], ps[:], bqv_sb[h][:])
                    ps = psmm.tile([128, nw], f32)
                    nc.tensor.matmul(ps[:], wk_sb[h], xb[:, h, :])
                    nc.vector.tensor_scalar_add(kT[:, csl], ps[:], bk_sb[h][:])
                    ps = psmm.tile([128, nw], f32)
                    nc.tensor.matmul(ps[:], wp_sb[h], peT_t[:, csl])
                    nc.scalar.copy(pT[:, csl], ps[:])
                # v in natural layout: v[s_tile, d] = xh @ Wv + bv
                for g in range(max(1, nt // 4)):
                    ps = psmm.tile([128, nw], f32)
                    gnt = min(4, nt)
                    for j in range(gnt):
                        nc.tensor.matmul(
                            ps[:, j * 128 : (j + 1) * 128],
                            xhT[min(g, nbands - 1)][:, h, j * 128 : (j + 1) * 128],
                            wv_sb[h],
                            start=True,
                            stop=False,
                        )
                        nc.tensor.matmul(
                            ps[:, j * 128 : (j + 1) * 128],
                            ones_row[:],
                            bvr_sb[h][:],
                            start=False,
                            stop=True,
                        )
                    nc.vector.tensor_copy(
                        v_sb[:, g * nw : g * nw + gnt * 128], ps[:, : gnt * 128]
                    )

                # ---- phase A: T = (q + v_bias) @ p^T  -> padded DRAM buffer ----
                tdram = drpool.tile([s, s + 1], bf16, tag="tshift")
                tflat = tdram[:].rearrange("a b -> (a b)")
                shview = tflat[s : s + s * s].rearrange("(a b) -> a b", b=s)
                for qt in range(nt):
                    tst = tstp.tile([128, s + 1], bf16)
                    nc.gpsimd.memset(tst[:, 0:1], 0.0)
                    for j in range(nn):
                        ps = psmm.tile([128, nw], f32)
                        nc.tensor.matmul(
                            ps[:], qvT[:, qt * 128 : (qt + 1) * 128], pT[:, j * nw : (j + 1) * nw]
                        )
                        nc.vector.tensor_copy(tst[:, 1 + j * nw : 1 + (j + 1) * nw], ps[:])
                    nc.sync.dma_start(tdram[qt * 128 : (qt + 1) * 128, :], tst[:])

                # ---- phase B: scores + softmax + transpose ----
                attnT = attp.tile([128, nt, s], bf16, tag="attnT")
                for qt in range(nt):
                    sh = shp.tile([128, s], bf16)
                    nc.sync.dma_start(sh[:], shview[qt * 128 : (qt + 1) * 128, :])
                    sc = scp.tile([128, s], bf16)
                    pss = []
                    for j in range(nn):
                        ps = psmm.tile([128, nw], f32, tag="ps", name=f"ps_{h}_{qt}_{j}")
                        nc.tensor.matmul(
                            ps[:],
                            quT[:, qt * 128 : (qt + 1) * 128],
                            kT[:, j * nw : (j + 1) * nw],
                            start=True,
                            stop=False,
                        )
                        pss.append(ps)
                    rsa = colp.tile([128, 4], f32, tag="rsa")
                    for j in range(nn):
                        nc.tensor.matmul(
                            pss[j][:],
                            ident_sb[:],
                            sh[:, j * nw : (j + 1) * nw],
                            start=False,
                            stop=True,
                        )
                        nc.scalar.activation(
                            sc[:, j * nw : (j + 1) * nw],
                            pss[j][:],
                            AF.Exp,
                            scale=inv_sqrt_e,
                            accum_out=rsa[:, j : j + 1],
                        )
                    rs = colp.tile([128, 1], f32, tag="rs")
                    nc.vector.reduce_sum(rs[:], rsa[:, :nn], axis=mybir.AxisListType.X)
                    rr = colp.tile([128, 1], f32, tag="rr")
                    nc.vector.reciprocal(rr[:], rs[:])
                    nc.vector.tensor_scalar_mul(sc[:], sc[:], rr[:])
                    nc.sync.dma_start_transpose(
                        attnT[:, :, qt * 128 : (qt + 1) * 128], sc[:]
                    )

                # ---- phase PV: ctxT = v^T @ attn^T (two half-width passes) ----
                ctxT = hbig2.tile([Dh, s], bf16, tag="ctxT")
                hwid = max(nw, s // 2)
                for half in range(max(1, s // hwid)):
                    c0 = half * hwid
                    ctx_ps = psctx.tile([128, hwid], f32)
                    for kc in range(nt):
                        for j in range(hwid // nw):
                            nc.tensor.matmul(
                                ctx_ps[:, j * nw : (j + 1) * nw],
                                v_sb[:, kc * 128 : (kc + 1) * 128],
                                attnT[:, kc, c0 + j * nw : c0 + (j + 1) * nw],
                                start=(kc == 0),
                                stop=(kc == nt - 1),
                            )
                    nc.vector.tensor_copy(ctxT[:, c0 : c0 + hwid], ctx_ps[:])
                ctxT_sb.append(ctxT)

            # ---- out-projection: out[s,:] = sum_h ctxT_h^T @ Wo_h ----
            for st in range(nt):
                out_t = outp.tile([128, E], f32)
                for j in range(ne):
                    ops = psmm.tile([128, 512], f32, tag="ps")
                    for h in range(HPC):
                        nc.tensor.matmul(
                            ops[:],
                            ctxT_sb[h][:, st * 128 : (st + 1) * 128],
                            wo_sb[h][:, j * 512 : (j + 1) * 512],
                            start=(h == 0),
                            stop=(h == HPC - 1),
                        )
                    nc.scalar.copy(out_t[:, j * 512 : (j + 1) * 512], ops[:])
                nc.sync.dma_start(out_ext[st * 128 : (st + 1) * 128, :], out_t[:])

    return nc


def _pos_encoding(s, e):
    pos = np.arange(s, dtype=np.float32)[:, None]
    div = np.exp(
        np.arange(0, e, 2, dtype=np.float32) * (-math.log(10000.0) / e)
    ).astype(np.float32)
    pe = np.zeros((s, e), dtype=np.float32)
    pe[:, 0::2] = np.sin(pos * div)
    pe[:, 1::2] = np.cos(pos * div)
    return pe


def make_core_inputs(inputs, s=S):
    """Host-side prep: fold LN gamma/beta into per-head weights, permute the
    input feature blocks so each core's two heads occupy columns 0:256."""
    x = np.asarray(inputs["inputs"], np.float32)
    gam = np.asarray(inputs["ln_gamma"], np.float32)
    bet = np.asarray(inputs["ln_beta"], np.float32)
    Wq = np.asarray(inputs["Wq"], np.float32)
    bq = np.asarray(inputs["bq"], np.float32)
    Wk = np.asarray(inputs["Wk"], np.float32)
    bk = np.asarray(inputs["bk"], np.float32)
    Wv = np.asarray(inputs["Wv"], np.float32)
    bv = np.asarray(inputs["bv"], np.float32)
    Wp = np.asarray(inputs["Wp"], np.float32)
    u_bias = np.asarray(inputs["u_bias"], np.float32)
    v_bias = np.asarray(inputs["v_bias"], np.float32)

    pe = _pos_encoding(s, E).reshape(s, H, Dh)

    in_maps = []
    Dh_ = Dh
    for c in range(NCORES):
        b = c // (NCORES // B)
        h0 = HPC * (c % (NCORES // B))
        heads = [h0 + i for i in range(HPC)]
        order = heads + [j for j in range(H) if j not in heads]
        x_c = np.ascontiguousarray(
            x[b, :s].reshape(s, H, Dh)[:, order, :].reshape(s, E)
        ).astype(BF)
        w4 = np.zeros((HPC, 4, Dh, Dh), BF)
        bqu = np.zeros((HPC, Dh), np.float32)
        bqv = np.zeros((HPC, Dh), np.float32)
        bkk = np.zeros((HPC, Dh), np.float32)
        bvr = np.zeros((HPC, Dh), BF)
        peT = np.zeros((HPC, Dh, s), BF)
        wo = np.zeros((HPC, Dh, E), BF)
        for i, hh in enumerate(heads):
            g = gam[hh * Dh : (hh + 1) * Dh]
            be = bet[hh * Dh : (hh + 1) * Dh]
            w4[i, 0] = (g[:, None] * Wq).astype(BF)
            w4[i, 1] = (g[:, None] * Wk).astype(BF)
            w4[i, 2] = (g[:, None] * Wv).astype(BF)
            w4[i, 3] = Wp.astype(BF)
            bqp = bq + be @ Wq
            bkp = bk + be @ Wk
            bvp = bv + be @ Wv
            bqu[i] = bqp + u_bias[hh]
            bqv[i] = bqp + v_bias[hh]
            bkk[i] = bkp
            bvr[i] = bvp.astype(BF)
            peT[i] = pe[:, hh, :].T.astype(BF)
            wo[i] = np.asarray(inputs["Wo"], np.float32)[
                hh * Dh : (hh + 1) * Dh, :
            ].astype(BF)
        in_maps.append(
            {
                "x": x_c,
                "ident": np.eye(Dh, dtype=BF),
                "w": w4,
                "bqu": bqu,
                "bqv": bqv,
                "bk": bkk,
                "bvrow": bvr,
                "peT": peT,
                "wo": wo,
            }
        )
    return in_maps


_PROG = {}


def _program(s):
    if s not in _PROG:
        _PROG[s] = _build_program(s)
    return _PROG[s]


def kernel(**inputs):
    from concourse.bass_utils import run_bass_kernel_spmd

    nc = _program(S)
    in_maps = make_core_inputs(inputs, S)
    res = run_bass_kernel_spmd(nc, in_maps, list(range(NCORES)))
    kernel.last_results = res
    out = np.zeros((B, S, E), np.float32)
    for c in range(NCORES):
        out[c // (NCORES // B)] += np.asarray(res.results[c]["out"], np.float32)
    out += np.asarray(inputs["bo"], np.float32)
    return out
